# revision 66
# baseline (speedup 1.0000x reference)
"""Trainium2 Bass kernel for nn_CFTLayer1d (Chebyshev filter layer), v3.

Data-parallel over batch: 1 batch item per NeuronCore (8 cores).

v3 design: 32-op "dirty product" DAG + PE mode sums.
  Per core x [C=64, L=65536]; per (segment s, quarter q) one tile
  [128 p (g-chunks), 64 c x 32 i] so per-(c,s) sums are partition-dim
  contractions done by ones-indicator matmuls into PSUM (v2's trick).
  Instead of v2's Chebyshev chains/ladders (40 ops/tile), v3 computes one
  stream per polynomial degree 1..32 (32 ops/tile), each a single plain
  product or activation-square of earlier streams:
    t1 (TSP), u2/u4/u8/u16 = doubling squares (ACT), odd degrees = products
    u{2^k}*a_r (DVE), even degrees = anchor products (DVE/Pool) + 3 squares
    (ACT). Host-side Chebyshev polynomial algebra computes each stream's
    exact expansion; the R-matrix solve folds weights so the device only
    needs the 32 raw sums + a tiny per-(c,s) dot product.
  Engine balance per tile: DVE 19 TT + t1 + min-reduce (~26us),
  ACT 7 squares (~15us), Pool 5 TT + max-reduce (~24us), PE 32 mode
  sums (~27.6us, the roofline). Out tiles are per-segment broadcast
  builds (1 ACT op) + 8 half-partition DMAs.
  tanh omitted on device: |rho| <= ~1.4e-3 so tanh(x)=x to ~1e-9 abs.
"""
import sys
import numpy as np
import numpy.polynomial.chebyshev as _cheb

for p in ("/opt/trn_rl_repo", "/opt/trn_rl_repo/concourse"):
    if p not in sys.path:
        sys.path.insert(0, p)

import concourse.bass as bass
import concourse.bacc as bacc
import concourse.tile as tile
from concourse import mybir
from concourse.bass_utils import run_bass_kernel_spmd

# Problem constants (hardcoded per contract)
B, C, L = 8, 64, 65536
S, M, O = 4, 32, 64
G = L // S                 # 16384 segment length
NCORES = 8
NQ = 4                     # quarter tiles per segment
FT = 2048                  # tile free size = 64 c * 32 i
NRAW = 32

F32 = mybir.dt.float32
F16 = mybir.dt.float16
AX = mybir.AxisListType
OP = mybir.AluOpType
AF = mybir.ActivationFunctionType

RT2 = float(np.sqrt(2.0))

# ---------------------------------------------------------------------------
# DAG: one stream per degree 1..32. kind: tsp | sq(ACT) | dve(TT) | pool(TT)
# Emission order = list order = PSUM row index.
# ---------------------------------------------------------------------------
_GRID = np.cos(np.linspace(0, np.pi, 4097))


def build_dag():
    """Returns ops list [(name, kind, src1, src2_or_scalebias)], R [32,33]."""
    exp = {"t1": np.array([0.0, 1.0])}
    ops = [("t1", "tsp", None, None)]

    def sq(nm, src):
        v = _cheb.chebval(_GRID, exp[src])
        lo, hi = float(v.min()), float(v.max())
        s_ = 2.0 * RT2 / (hi - lo)
        b_ = -RT2 * (hi + lo) / (hi - lo)
        p_ = s_ * exp[src].copy()
        p_[0] += b_
        exp[nm] = _cheb.chebmul(p_, p_)
        ops.append((nm, "sq", src, (s_, b_)))

    def tt(nm, a, b, eng="dve"):
        exp[nm] = _cheb.chebmul(exp[a], exp[b])
        ops.append((nm, eng, a, b))

    sq("u2", "t1")
    tt("a3", "u2", "t1")
    sq("u4", "u2")
    tt("a5", "u4", "t1")
    tt("a7", "u4", "a3")
    tt("e6", "u2", "u4", "pool")
    sq("u8", "u4")
    tt("a9", "u8", "t1")
    tt("a11", "u8", "a3")
    tt("a13", "u8", "a5")
    tt("a15", "u8", "a7")
    tt("e10", "u2", "u8")
    tt("e12", "u4", "u8", "pool")
    tt("e14", "u2", "e12")
    sq("u16", "u8")
    tt("a17", "u16", "t1")
    tt("a19", "u16", "a3")
    tt("a21", "u16", "a5")
    tt("a23", "u16", "a7")
    tt("a25", "u16", "a9")
    tt("a27", "u16", "a11")
    tt("a29", "u16", "a13")
    tt("a31", "u16", "a15")
    sq("e18", "a9")
    tt("e20", "u4", "u16", "pool")
    tt("e22", "e6", "u16", "pool")
    sq("e24", "e12")
    sq("e26", "a13")
    sq("e28", "e14")
    tt("e30", "e14", "u16")
    sq("e32", "u16")
    assert len(ops) == NRAW
    degs = sorted(len(exp[nm]) - 1 for (nm, _, _, _) in ops)
    assert degs == list(range(1, 33)), degs

    R = np.zeros((NRAW, 33))
    for i, (nm, _, _, _) in enumerate(ops):
        c = exp[nm]
        R[i, 32] = c[0]
        for k in range(1, len(c)):
            R[i, k - 1] = c[k]
    return ops, R


def host_weight_transform(W: np.ndarray, R: np.ndarray):
    """W [S,M,C,O] -> q [32, S*C] f32, off [C, S] f32 such that
    rho[c,s] = sum_i q[i, s*C+c] * RawSum_i[c,s] + off[c,s]."""
    Wbar = W.astype(np.float64).mean(axis=3)          # [S, M, C]
    alpha = np.transpose(Wbar, (2, 0, 1))             # [C, S, M]
    beta = np.zeros((C, S, 32))
    for k in range(1, 33):
        gm = 2.0 if k == 1 else 1.0
        t = gm * alpha[:, :, k - 1] if k - 1 <= M - 1 else 0.0
        if k + 1 <= M - 1:
            t = t + alpha[:, :, k + 1]
        beta[:, :, k - 1] = t / (2.0 * G)
    off = alpha[:, :, 1] / 2.0                        # [C, S]

    Rs = R[:, :32]
    Rg = R[:, 32]
    qall = np.linalg.solve(Rs.T, beta.reshape(-1, 32).T).T  # [(C*S), 32]
    qall = qall.reshape(C, S, NRAW)
    off2 = off - (qall @ Rg) * G                      # [C, S]

    q_dev = np.zeros((NRAW, S, C), dtype=np.float32)
    for s in range(S):
        q_dev[:, s, :] = qall[:, s, :].T
    return q_dev.reshape(NRAW, S * C), off2.astype(np.float32)


# ---------------------------------------------------------------------------
# device kernel
# ---------------------------------------------------------------------------
def build_kernel():
    ops, _ = build_dag()
    nc = bacc.Bacc("TRN2", target_bir_lowering=False, num_devices=NCORES)

    x_in = nc.dram_tensor("x", [C, L], F32, kind="ExternalInput")
    q_in = nc.dram_tensor("q", [NRAW, S * C], F32, kind="ExternalInput")
    off_in = nc.dram_tensor("off", [C, S], F32, kind="ExternalInput")
    out = nc.dram_tensor("out", [C, L], F32, kind="ExternalOutput")

    # out layout: partition p = h*64+c holds out[c, h*32768:(h+1)*32768];
    # segment s lives in half h = s // 2 at chunk t = (s % 2) * 8 + 0..7.
    outr = out[:, :].rearrange("c (h l) -> c h l", h=2).transpose([1, 0, 2])

    def src_ap(s, q):
        # [128 p, 64 c, 32 i]: x[c, s*G + 4096*q + 32*p + i]
        return (x_in[:, G * s + 4096 * q:G * s + 4096 * (q + 1)]
                .rearrange("c (p i) -> p c i", p=128))

    def src_ap_cg(s, g):
        # cheap-DMA minmax layout (512B runs): [128 p, 16 c, 128 i]
        return (x_in[16 * g:16 * (g + 1), G * s:G * (s + 1)]
                .rearrange("c (p i) -> p c i", p=128))

    with tile.TileContext(nc) as tc:
        with (
            tc.tile_pool(name="xb", bufs=5) as xb_pool,
            tc.tile_pool(name="anch", bufs=2) as an_pool,    # u2,u4,u8,u16,e6,e10,e12,e14
            tc.tile_pool(name="prod", bufs=1) as pr_pool,    # t1,a3..a15
            tc.tile_pool(name="junk", bufs=4) as j_pool,     # leaf streams
            tc.tile_pool(name="ot", bufs=2) as o_pool,       # out broadcast
            tc.tile_pool(name="small", bufs=1) as sm_pool,
            tc.tile_pool(name="ps", bufs=2, space="PSUM") as ps_pool,
            tc.tile_pool(name="dram", bufs=1, space="DRAM") as dram_pool,
        ):
            # indicator lhsT matrices: E[:, k, j] = (j == k); matmul with
            # lhsT = E[:, k, :] lands the ones^T row-sum on PSUM row k while
            # accumulating zeros into the other 31 rows.
            Emat = sm_pool.tile([128, NRAW, NRAW], F16)
            nc.vector.memset(Emat[:], 0.0)
            for k in range(NRAW):
                nc.vector.memset(Emat[:, k, k:k + 1], 1.0)
            ones32f = sm_pool.tile([NRAW, 1], F32)
            nc.vector.memset(ones32f[:], 1.0)
            zeros16 = sm_pool.tile([128, 512], F16)
            nc.vector.memset(zeros16[:], 0.0)
            zeros_ot = sm_pool.tile([64, 2 * FT], F16)
            nc.vector.memset(zeros_ot[:], 0.0)
            q_sb = sm_pool.tile([NRAW, S * C], F32)
            nc.sync.dma_start(q_sb[:], q_in[:, :])
            off_sb = sm_pool.tile([C, S], F32)
            nc.sync.dma_start(off_sb[:], off_in[:, :])
            SCB = sm_pool.tile([128, 2 * S], F32)    # a_s at col s, b_s at S+s
            Sacc = sm_pool.tile([NRAW, S, C], F32)

            # bias const tiles for ACT squares (floats need const APs)
            sq_biases = sorted({round(float(s2[1]), 9) for (_, k, _, s2)
                                in ops if k == "sq" and s2[1] != 0.0})
            bias_t = {}
            for bi, bv in enumerate(sq_biases):
                bt = sm_pool.tile([128, 1], F32, name=f"bias{bi}")
                nc.vector.memset(bt[:], bv)
                bias_t[bv] = bt

            # ---------------- phase A: min/max + collective ----------------
            MM = {}
            x_tiles = {}

            def load_tile(s, q):
                xb = xb_pool.tile([128, C, 32], F32, tag="xb")
                nc.sync.dma_start(xb[:], src_ap(s, q))
                x_tiles[(s, q)] = xb

            MN = {}

            def minmax_dve(s, q, tile=None):
                # per-partition min -> negate; Pool folds it later
                if q == 0:
                    MM[s] = sm_pool.tile([1, 2, NQ], F32, tag=f"MM{s}",
                                         name=f"MM{s}")
                xb = tile if tile is not None else x_tiles[(s, q)]
                xbf = xb[:].rearrange("p c i -> p (c i)")
                mn = sm_pool.tile([128, 1], F32, tag="mn", bufs=4)
                nc.vector.tensor_reduce(mn[:], xbf, AX.X, OP.min)
                nc.vector.tensor_scalar_mul(mn[:], mn[:], -1.0)
                MN[(s, q)] = mn

            def minmax_pool(s, q, tile=None):
                # MM[., 0, q] = tile max; MM[., 1, q] = -min (as max(-x))
                xb = tile if tile is not None else x_tiles[(s, q)]
                xbf = xb[:].rearrange("p c i -> p (c i)")
                nc.gpsimd.tensor_reduce(MM[s][0:1, 0, q:q + 1], xbf,
                                        AX.XYZWC, OP.max)
                nc.gpsimd.tensor_reduce(MM[s][0:1, 1, q:q + 1],
                                        MN.pop((s, q))[:], AX.XYZWC, OP.max)

            def phaseA_fold(s):
                dq = nc.sync
                g = nc.gpsimd
                mm = MM[s]
                M1 = sm_pool.tile([1, 2], F32, tag=f"M1{s}", name=f"M1{s}")
                g.tensor_reduce(M1[0:1, 0:1], mm[0:1, 0:1, :], AX.XYZWC, OP.max)
                g.tensor_reduce(M1[0:1, 1:2], mm[0:1, 1:2, :], AX.XYZWC, OP.max)
                cc_in = dram_pool.tile([1, 2], F32, tag=f"cci{s}")
                cc_out = dram_pool.tile([8, 2], F32, tag=f"cco{s}")
                dq.dma_start(cc_in[:], M1[:])
                nc.gpsimd.collective_compute(
                    "AllGather", OP.bypass,
                    replica_groups=[list(range(NCORES))],
                    ins=[cc_in.opt()], outs=[cc_out.opt()])
                GRt = sm_pool.tile([1, 16], F32, tag=f"GR{s}", name=f"GR{s}")
                dq.dma_start(GRt[:], cc_out[:, :].rearrange("r j -> (r j)"))
                return GRt

            def phaseA_calc(s, GRt):
                # xn = a*x + b; a = 2/(max-min), b = (negmin-max)/(max-min)
                v = nc.vector
                GRm = sm_pool.tile([1, 2], F32, tag=f"GRm{s}", name=f"GRm{s}")
                v.tensor_reduce(GRm[:], GRt[:].rearrange("o (r j) -> o j r", r=8),
                                AX.X, OP.max)
                den = sm_pool.tile([1, 1], F32, tag=f"den{s}")
                v.tensor_add(den[:], GRm[:, 0:1], GRm[:, 1:2])
                rden = sm_pool.tile([1, 1], F32, tag=f"rden{s}")
                v.reciprocal(rden[:], den[:])
                S2 = sm_pool.tile([1, 2], F32, tag=f"S2{s}")
                v.tensor_scalar_mul(S2[:, 0:1], rden[:], 2.0)
                dif = sm_pool.tile([1, 1], F32, tag=f"dif{s}")
                v.tensor_sub(dif[:], GRm[:, 1:2], GRm[:, 0:1])
                v.tensor_mul(S2[:, 1:2], dif[:], rden[:])
                nc.gpsimd.partition_broadcast(SCB[:, s:s + 1], S2[:, 0:1])
                nc.gpsimd.partition_broadcast(SCB[:, S + s:S + s + 1],
                                              S2[:, 1:2])

            # ---------------- phase B: streams + PE sums ----------------
            # Software-pipelined "phase skew": P1 = ops[0:16] (t1..u16),
            # P2 = ops[16:32] (u16-dependent leaves). Window k emits the ACT
            # anchor chain of tile k, then P2 of tile k-1 (inputs all ready),
            # then the rest of P1 of tile k. PE consumes P2(k-1) mms while
            # tile k's chain fills, so it never starves.
            NP1 = 16
            assert ops[NP1 - 1][0] == "u16"
            tile_streams = {}
            seg_ps = {}

            # per-tag buffer counts (cross-window readers need 2)
            TAG_BUFS = {"t1": 2, "a3": 2, "a5": 2, "a7": 2, "a9": 2,
                        "a11": 2, "a13": 2, "a15": 2,
                        "u2": 2, "u4": 2, "u8": 2, "u16": 2,
                        "e12": 2, "e6": 1, "e14": 2}

            def stream_tile(nm):
                if (nm.startswith("a") and int(nm[1:]) >= 17) or \
                   nm in ("e10", "e18", "e20", "e22", "e24", "e26", "e28",
                          "e30", "e32"):
                    return j_pool.tile([128, FT], F16, tag="junk", name="junk")
                pool = an_pool if nm[0] in "ue" else pr_pool
                return pool.tile([128, FT], F16, tag=nm, bufs=TAG_BUFS[nm],
                                 name=nm)

            def mm(ps, idx, v, start=False, stop=False):
                for ch in range(4):
                    nc.tensor.matmul(
                        ps[0:NRAW, 512 * ch:512 * (ch + 1)],
                        Emat[:, idx, :], v[:, 512 * ch:512 * (ch + 1)],
                        start=start, stop=stop,
                        skip_group_check=True)

            def emit_op(st, s, nm, kind, s1, s2, xbf=None):
                t = stream_tile(nm)
                if kind == "tsp":
                    nc.scalar.activation(t[:], xbf, AF.Identity,
                                         bias=SCB[:, S + s:S + s + 1],
                                         scale=SCB[:, s:s + 1])
                elif kind == "sq":
                    bv = round(float(s2[1]), 9)
                    bias_ap = bias_t[bv][:, 0:1] if bv != 0.0 else 0.0
                    nc.scalar.activation(t[:], st[s1][:], AF.Square,
                                         bias=bias_ap, scale=float(s2[0]))
                else:
                    eng = nc.vector if kind == "dve" else nc.gpsimd
                    eng.tensor_mul(t[:], st[s1][:], st[s2][:])
                st[nm] = t
                return t

            def emit_drain(s):
                nc.vector.tensor_reduce(
                    Sacc[:, s, :],
                    seg_ps[s][0:NRAW].rearrange("k (c i) -> k c i", c=C),
                    AX.X, OP.add)

            OP_IDX = {nm: i for i, (nm, _, _, _) in enumerate(ops)}
            # matmul emission order ~ stream-readiness order: interleaves
            # prev-tile P2 leaves ("p") with current-tile chain ("c") and
            # rest-of-P1 ("r") so the in-order PE never starves.
            MM_ORDER = [
                ("p", "a17"), ("c", "t1"), ("p", "a19"), ("p", "a21"),
                ("c", "u2"), ("p", "e20"), ("p", "a23"), ("p", "a25"),
                ("c", "u4"), ("p", "a27"), ("p", "a29"), ("c", "u8"),
                ("p", "a31"), ("p", "e30"), ("p", "e22"), ("c", "u16"),
                ("r", "a3"), ("r", "a5"), ("p", "e18"), ("r", "a7"),
                ("r", "e6"), ("p", "e24"), ("r", "a9"), ("r", "a11"),
                ("p", "e26"), ("r", "a13"), ("r", "e12"), ("r", "a15"),
                ("p", "e28"), ("r", "e10"), ("r", "e14"), ("p", "e32"),
            ]
            assert len(MM_ORDER) == 32

            # ---------------- combine + out ----------------
            def combine(s, split=False):
                # rho[c] = sum_i q[i,c]*Sacc[i,c] via tiny f32 matmul into the
                # spare PSUM partitions (64..127) of the segment accumulator.
                prod_ = sm_pool.tile([NRAW, C], F32, tag=f"pr{s}")
                nc.vector.tensor_mul(prod_[:], Sacc[:, s, :],
                                     q_sb[:, C * s:C * (s + 1)])
                ps = seg_ps[s]
                nc.tensor.matmul(ps[64:128, s:s + 1], prod_[:], ones32f[:],
                                 start=True, stop=True, skip_group_check=True)
                rt = sm_pool.tile([64, 1], F32, tag=f"rt{s}")
                nc.vector.tensor_add(rt[:], ps[64:128, s:s + 1],
                                     off_sb[:, s:s + 1])
                # broadcast build: [64, 4096] f32 = rho bias (scale=0 ignores
                # the input, so read the output tile itself)
                ot = o_pool.tile([64, 2 * FT], F32, tag="ot", bufs=1)
                if split:
                    # latency-critical tail: build halves on ACT + DVE
                    nc.scalar.activation(ot[:, 0:FT], zeros_ot[:, 0:FT],
                                         AF.Identity,
                                         bias=rt[:, 0:1], scale=0.0)
                    nc.vector.tensor_scalar(ot[:, FT:], zeros_ot[:, FT:], 0.0,
                                            rt[:, 0:1], OP.mult, OP.add)
                else:
                    nc.scalar.activation(ot[:], zeros_ot[:], AF.Identity,
                                         bias=rt[:, 0:1], scale=0.0)
                return ot

            def out_dma(s, ot, t, eng=None):
                h, tt_ = s // 2, (s % 2) * 4 + t
                (eng or nc.sync).dma_start(
                    outr[h, :, bass.ts(tt_, 2 * FT)], ot[:])

            # ---------------- schedule ----------------
            # head minmax via cheap-DMA layout tiles (512B runs, ~2x faster
            # loads), then reload segment 0 in compute layout behind the
            # collective; head tiles rotate through the same xb pool bufs.
            s0 = 0
            for g in range(NQ):
                hx = xb_pool.tile([128, C, 32], F32, tag="xb",
                                  name=f"hx{g}")
                nc.sync.dma_start(
                    hx[:].rearrange("p c i -> p (c i)"), src_ap_cg(s0, g))
                minmax_dve(s0, g, tile=hx)
                minmax_pool(s0, g, tile=hx)
            GR0 = phaseA_fold(s0)
            for qq in range(NQ):
                load_tile(s0, qq)

            # PE p-state warmup while the collective is in flight
            NWARM = 42
            ps_w = ps_pool.tile([128, FT], F32, tag="ps", name="ps_w")
            for wi in range(NWARM):
                for ch in range(4):
                    nc.tensor.matmul(
                        ps_w[0:NRAW, 512 * ch:512 * (ch + 1)],
                        Emat[:, 0, :], zeros16[:, :],
                        start=(wi == 0), stop=(wi == NWARM - 1),
                        skip_group_check=True)
            phaseA_calc(s0, GR0)

            GR_next = [None]
            out_work = {}
            NT = S * NQ

            pending_drain = [None]
            pending_combine = [None]
            HOOK_AT = 15   # after all P2-DVE entries in MM_ORDER

            for k in range(NT + 1):
                s, q = divmod(k, NQ) if k < NT else (None, None)
                sp, qp = divmod(k - 1, NQ) if k >= 1 else (None, None)
                s_next = (s + 1 if s is not None and s + 1 < S else None)

                def dve_aux():
                    # DVE aux early inside the P2-DVE run (PE has buffer
                    # there); keeps the DVE second half free for P1 streams
                    if s_next is not None:
                        if q == 0:
                            minmax_dve(s_next, 0)
                        elif q == 1:
                            minmax_dve(s_next, 2)
                        elif q == 2:
                            minmax_dve(s_next, 3)

                def dve_aux2():
                    if pending_drain[0] is not None:
                        sd = pending_drain[0]
                        pending_drain[0] = None
                        emit_drain(sd)
                        pending_combine[0] = sd

                def hook():
                    if pending_combine[0] is not None:
                        sc = pending_combine[0]
                        pending_combine[0] = None
                        out_work[sc] = combine(sc)
                    if s_next is not None and q == 3:
                        phaseA_calc(s_next, GR_next[0])
                    # out DMAs for the previously combined segment
                    if s is not None and s > 0 and q in (2, 3) \
                            and (s - 1) in out_work:
                        otp = out_work[s - 1]
                        for t in range(2):
                            out_dma(s - 1, otp, (q - 2) * 2 + t)

                def tail_aux():
                    if s_next is not None:
                        if q == 0:
                            minmax_dve(s_next, 1)
                            minmax_pool(s_next, 0)
                            minmax_pool(s_next, 1)
                        elif q == 1:
                            minmax_pool(s_next, 2)
                        elif q == 2:
                            minmax_pool(s_next, 3)
                            GR_next[0] = phaseA_fold(s_next)

                # --- loads first (independent SP-queue work) ---
                if s_next is not None:
                    if q == 0:
                        load_tile(s_next, 0)
                        load_tile(s_next, 1)
                    elif q == 1:
                        load_tile(s_next, 2)
                        load_tile(s_next, 3)

                # --- (op, matmuls) pairs in readiness order ---
                if k < NT:
                    if q == 0:
                        seg_ps[s] = ps_pool.tile([128, FT], F32, tag="ps",
                                                 name=f"ps{s}")
                    xb = x_tiles.pop((s, q))
                    xbf = xb[:].rearrange("p c i -> p (c i)")
                    tile_streams[(s, q)] = {}
                for mi, (src, nm) in enumerate(MM_ORDER):
                    if mi == 4:
                        dve_aux()
                    if mi == 9:
                        dve_aux2()
                    if mi == HOOK_AT:
                        hook()

                    i = OP_IDX[nm]
                    _, kind, s1, s2 = ops[i]
                    if src == "p":
                        if k >= 1:
                            st = tile_streams[(sp, qp)]
                            t = emit_op(st, sp, nm, kind, s1, s2)
                            mm(seg_ps[sp], i, t,
                               stop=(nm == "e32" and qp == NQ - 1))
                    else:
                        if k < NT:
                            st = tile_streams[(s, q)]
                            t = emit_op(st, s, nm, kind, s1, s2, xbf=xbf)
                            mm(seg_ps[s], i, t,
                               start=(nm == "t1" and q == 0))
                            if k == 0:
                                # window 0 is production-paced: fillers that
                                # read the fresh stream execute in the gaps,
                                # keeping the PE p-state hot
                                for _ in range(3):
                                    nc.tensor.matmul(
                                        ps_w[0:NRAW, 0:512], Emat[:, 0, :],
                                        t[:, 0:512], start=True, stop=True,
                                        skip_group_check=True)
                if k < NT:
                    tail_aux()
                if k >= 1:
                    del tile_streams[(sp, qp)]
                    if qp == NQ - 1:
                        pending_drain[0] = sp

            # tail: drain + combine + outs of the last segment on 4 queues
            emit_drain(S - 1)
            ot = combine(S - 1, split=True)
            for t, eng in enumerate((nc.sync, nc.gpsimd, nc.scalar, nc.sync)):
                out_dma(S - 1, ot, t, eng=eng)

    nc.compile()
    return nc


_NC_CACHE = {}


def _get_nc():
    if "nc" not in _NC_CACHE:
        _NC_CACHE["nc"] = build_kernel()
    return _NC_CACHE["nc"]


def kernel(x: np.ndarray, chebyshev_weights: np.ndarray, **run_kwargs) -> np.ndarray:
    x = np.ascontiguousarray(np.asarray(x, dtype=np.float32))
    W = np.asarray(chebyshev_weights, dtype=np.float32)
    assert x.shape == (B, C, L), x.shape
    _, R = build_dag()
    q_dev, off_dev = host_weight_transform(W, R)

    nc = _get_nc()
    in_maps = [
        {"x": x[b], "q": q_dev, "off": off_dev}
        for b in range(NCORES)
    ]
    res = run_bass_kernel_spmd(nc, in_maps, core_ids=list(range(NCORES)),
                               **run_kwargs)
    out = np.stack([res.results[b]["out"] for b in range(NCORES)], axis=0)
    kernel.last_results = res
    return out


# revision 68
# speedup vs baseline: 1.0344x; 1.0344x over previous
"""Trainium2 Bass kernel for nn_CFTLayer1d (Chebyshev filter layer), v3.

Data-parallel over batch: 1 batch item per NeuronCore (8 cores).

v3 design: 32-op "dirty product" DAG + PE mode sums.
  Per core x [C=64, L=65536]; per (segment s, quarter q) one tile
  [128 p (g-chunks), 64 c x 32 i] so per-(c,s) sums are partition-dim
  contractions done by ones-indicator matmuls into PSUM (v2's trick).
  Instead of v2's Chebyshev chains/ladders (40 ops/tile), v3 computes one
  stream per polynomial degree 1..32 (32 ops/tile), each a single plain
  product or activation-square of earlier streams:
    t1 (TSP), u2/u4/u8/u16 = doubling squares (ACT), odd degrees = products
    u{2^k}*a_r (DVE), even degrees = anchor products (DVE/Pool) + 3 squares
    (ACT). Host-side Chebyshev polynomial algebra computes each stream's
    exact expansion; the R-matrix solve folds weights so the device only
    needs the 32 raw sums + a tiny per-(c,s) dot product.
  Engine balance per tile: DVE 19 TT + t1 + min-reduce (~26us),
  ACT 7 squares (~15us), Pool 5 TT + max-reduce (~24us), PE 32 mode
  sums (~27.6us, the roofline). Out tiles are per-segment broadcast
  builds (1 ACT op) + 8 half-partition DMAs.
  tanh omitted on device: |rho| <= ~1.4e-3 so tanh(x)=x to ~1e-9 abs.
"""
import sys
import numpy as np
import numpy.polynomial.chebyshev as _cheb

for p in ("/opt/trn_rl_repo", "/opt/trn_rl_repo/concourse"):
    if p not in sys.path:
        sys.path.insert(0, p)

import concourse.bass as bass
import concourse.bacc as bacc
import concourse.tile as tile
from concourse import mybir
from concourse.bass_utils import run_bass_kernel_spmd

# Problem constants (hardcoded per contract)
B, C, L = 8, 64, 65536
S, M, O = 4, 32, 64
G = L // S                 # 16384 segment length
NCORES = 8
NQ = 4                     # quarter tiles per segment
FT = 2048                  # tile free size = 64 c * 32 i
NRAW = 32

F32 = mybir.dt.float32
F16 = mybir.dt.float16
AX = mybir.AxisListType
OP = mybir.AluOpType
AF = mybir.ActivationFunctionType

RT2 = float(np.sqrt(2.0))

# ---------------------------------------------------------------------------
# DAG: one stream per degree 1..32. kind: tsp | sq(ACT) | dve(TT) | pool(TT)
# Emission order = list order = PSUM row index.
# ---------------------------------------------------------------------------
_GRID = np.cos(np.linspace(0, np.pi, 4097))


def build_dag():
    """Returns ops list [(name, kind, src1, src2_or_scalebias)], R [32,33]."""
    exp = {"t1": np.array([0.0, 1.0])}
    ops = [("t1", "tsp", None, None)]

    def sq(nm, src):
        v = _cheb.chebval(_GRID, exp[src])
        lo, hi = float(v.min()), float(v.max())
        s_ = 2.0 * RT2 / (hi - lo)
        b_ = -RT2 * (hi + lo) / (hi - lo)
        p_ = s_ * exp[src].copy()
        p_[0] += b_
        exp[nm] = _cheb.chebmul(p_, p_)
        ops.append((nm, "sq", src, (s_, b_)))

    def tt(nm, a, b, eng="dve"):
        exp[nm] = _cheb.chebmul(exp[a], exp[b])
        ops.append((nm, eng, a, b))

    sq("u2", "t1")
    tt("a3", "u2", "t1")
    sq("u4", "u2")
    tt("a5", "u4", "t1")
    tt("a7", "u4", "a3")
    tt("e6", "u2", "u4", "pool")
    sq("u8", "u4")
    tt("a9", "u8", "t1")
    tt("a11", "u8", "a3")
    tt("a13", "u8", "a5")
    tt("a15", "u8", "a7")
    tt("e10", "u2", "u8")
    tt("e12", "u4", "u8", "pool")
    tt("e14", "u2", "e12")
    sq("u16", "u8")
    tt("a17", "u16", "t1")
    tt("a19", "u16", "a3")
    tt("a21", "u16", "a5")
    tt("a23", "u16", "a7")
    tt("a25", "u16", "a9")
    tt("a27", "u16", "a11")
    tt("a29", "u16", "a13")
    tt("a31", "u16", "a15")
    sq("e18", "a9")
    tt("e20", "u4", "u16", "pool")
    tt("e22", "e6", "u16", "pool")
    sq("e24", "e12")
    sq("e26", "a13")
    sq("e28", "e14")
    tt("e30", "e14", "u16")
    sq("e32", "u16")
    assert len(ops) == NRAW
    degs = sorted(len(exp[nm]) - 1 for (nm, _, _, _) in ops)
    assert degs == list(range(1, 33)), degs

    R = np.zeros((NRAW, 33))
    for i, (nm, _, _, _) in enumerate(ops):
        c = exp[nm]
        R[i, 32] = c[0]
        for k in range(1, len(c)):
            R[i, k - 1] = c[k]
    return ops, R


def host_weight_transform(W: np.ndarray, R: np.ndarray):
    """W [S,M,C,O] -> q [32, S*C] f32, off [C, S] f32 such that
    rho[c,s] = sum_i q[i, s*C+c] * RawSum_i[c,s] + off[c,s]."""
    Wbar = W.astype(np.float64).mean(axis=3)          # [S, M, C]
    alpha = np.transpose(Wbar, (2, 0, 1))             # [C, S, M]
    beta = np.zeros((C, S, 32))
    for k in range(1, 33):
        gm = 2.0 if k == 1 else 1.0
        t = gm * alpha[:, :, k - 1] if k - 1 <= M - 1 else 0.0
        if k + 1 <= M - 1:
            t = t + alpha[:, :, k + 1]
        beta[:, :, k - 1] = t / (2.0 * G)
    off = alpha[:, :, 1] / 2.0                        # [C, S]

    Rs = R[:, :32]
    Rg = R[:, 32]
    qall = np.linalg.solve(Rs.T, beta.reshape(-1, 32).T).T  # [(C*S), 32]
    qall = qall.reshape(C, S, NRAW)
    off2 = off - (qall @ Rg) * G                      # [C, S]

    q_dev = np.zeros((NRAW, S, C), dtype=np.float32)
    for s in range(S):
        q_dev[:, s, :] = qall[:, s, :].T
    return q_dev.reshape(NRAW, S * C), off2.astype(np.float32)


# ---------------------------------------------------------------------------
# device kernel
# ---------------------------------------------------------------------------
def build_kernel():
    ops, _ = build_dag()
    nc = bacc.Bacc("TRN2", target_bir_lowering=False, num_devices=NCORES)

    x_in = nc.dram_tensor("x", [C, L], F32, kind="ExternalInput")
    q_in = nc.dram_tensor("q", [NRAW, S * C], F32, kind="ExternalInput")
    off_in = nc.dram_tensor("off", [C, S], F32, kind="ExternalInput")
    out = nc.dram_tensor("out", [C, L], F32, kind="ExternalOutput")

    # out layout: partition p = h*64+c holds out[c, h*32768:(h+1)*32768];
    # segment s lives in half h = s // 2 at chunk t = (s % 2) * 8 + 0..7.
    outr = out[:, :].rearrange("c (h l) -> c h l", h=2).transpose([1, 0, 2])

    def src_ap(s, q):
        # [128 p, 64 c, 32 i]: x[c, s*G + 4096*q + 32*p + i]
        return (x_in[:, G * s + 4096 * q:G * s + 4096 * (q + 1)]
                .rearrange("c (p i) -> p c i", p=128))

    def src_ap_cg(s, g):
        # cheap-DMA minmax layout (512B runs): [128 p, 16 c, 128 i]
        return (x_in[16 * g:16 * (g + 1), G * s:G * (s + 1)]
                .rearrange("c (p i) -> p c i", p=128))

    with tile.TileContext(nc) as tc:
        with (
            tc.tile_pool(name="xb", bufs=5) as xb_pool,
            tc.tile_pool(name="anch", bufs=2) as an_pool,    # u2,u4,u8,u16,e6,e10,e12,e14
            tc.tile_pool(name="prod", bufs=1) as pr_pool,    # t1,a3..a15
            tc.tile_pool(name="junk", bufs=4) as j_pool,     # leaf streams
            tc.tile_pool(name="ot", bufs=2) as o_pool,       # out broadcast
            tc.tile_pool(name="small", bufs=1) as sm_pool,
            tc.tile_pool(name="ps", bufs=2, space="PSUM") as ps_pool,
            tc.tile_pool(name="dram", bufs=1, space="DRAM") as dram_pool,
        ):
            # indicator lhsT matrices: E[:, k, j] = (j == k); matmul with
            # lhsT = E[:, k, :] lands the ones^T row-sum on PSUM row k while
            # accumulating zeros into the other 31 rows.
            Emat = sm_pool.tile([128, NRAW, NRAW], F16)
            nc.vector.memset(Emat[:], 0.0)
            for k in range(NRAW):
                nc.vector.memset(Emat[:, k, k:k + 1], 1.0)
            ones32f = sm_pool.tile([NRAW, 1], F32)
            nc.vector.memset(ones32f[:], 1.0)
            zeros16 = sm_pool.tile([128, 512], F16)
            nc.vector.memset(zeros16[:], 0.0)
            zeros_ot = sm_pool.tile([64, 2 * FT], F16)
            nc.vector.memset(zeros_ot[:], 0.0)
            q_sb = sm_pool.tile([NRAW, S * C], F32)
            nc.sync.dma_start(q_sb[:], q_in[:, :])
            off_sb = sm_pool.tile([C, S], F32)
            nc.sync.dma_start(off_sb[:], off_in[:, :])
            SCB = sm_pool.tile([128, 2 * S], F32)    # a_s at col s, b_s at S+s
            Sacc = sm_pool.tile([NRAW, S, C], F32)

            # bias const tiles for ACT squares (floats need const APs)
            sq_biases = sorted({round(float(s2[1]), 9) for (_, k, _, s2)
                                in ops if k == "sq" and s2[1] != 0.0})
            bias_t = {}
            for bi, bv in enumerate(sq_biases):
                bt = sm_pool.tile([128, 1], F32, name=f"bias{bi}")
                nc.vector.memset(bt[:], bv)
                bias_t[bv] = bt

            # ---------------- phase A: min/max + collective ----------------
            MM = {}
            x_tiles = {}

            def load_tile(s, q):
                xb = xb_pool.tile([128, C, 32], F32, tag="xb")
                nc.sync.dma_start(xb[:], src_ap(s, q))
                x_tiles[(s, q)] = xb

            MN = {}

            def minmax_dve(s, q, tile=None):
                # per-partition min -> negate; Pool folds it later
                if q == 0:
                    MM[s] = sm_pool.tile([1, 2, NQ], F32, tag=f"MM{s}",
                                         name=f"MM{s}")
                xb = tile if tile is not None else x_tiles[(s, q)]
                xbf = xb[:].rearrange("p c i -> p (c i)")
                mn = sm_pool.tile([128, 1], F32, tag="mn", bufs=4)
                nc.vector.tensor_reduce(mn[:], xbf, AX.X, OP.min)
                nc.vector.tensor_scalar_mul(mn[:], mn[:], -1.0)
                MN[(s, q)] = mn

            def minmax_pool(s, q, tile=None):
                # MM[., 0, q] = tile max; MM[., 1, q] = -min (as max(-x))
                xb = tile if tile is not None else x_tiles[(s, q)]
                xbf = xb[:].rearrange("p c i -> p (c i)")
                nc.gpsimd.tensor_reduce(MM[s][0:1, 0, q:q + 1], xbf,
                                        AX.XYZWC, OP.max)
                nc.gpsimd.tensor_reduce(MM[s][0:1, 1, q:q + 1],
                                        MN.pop((s, q))[:], AX.XYZWC, OP.max)

            def phaseA_fold(s):
                dq = nc.sync
                g = nc.gpsimd
                mm = MM[s]
                M1 = sm_pool.tile([1, 2], F32, tag=f"M1{s}", name=f"M1{s}")
                g.tensor_reduce(M1[0:1, 0:1], mm[0:1, 0:1, :], AX.XYZWC, OP.max)
                g.tensor_reduce(M1[0:1, 1:2], mm[0:1, 1:2, :], AX.XYZWC, OP.max)
                cc_in = dram_pool.tile([1, 2], F32, tag=f"cci{s}")
                cc_out = dram_pool.tile([8, 2], F32, tag=f"cco{s}")
                dq.dma_start(cc_in[:], M1[:])
                nc.gpsimd.collective_compute(
                    "AllGather", OP.bypass,
                    replica_groups=[list(range(NCORES))],
                    ins=[cc_in.opt()], outs=[cc_out.opt()])
                GRt = sm_pool.tile([1, 16], F32, tag=f"GR{s}", name=f"GR{s}")
                dq.dma_start(GRt[:], cc_out[:, :].rearrange("r j -> (r j)"))
                return GRt

            def phaseA_calc(s, GRt):
                # xn = a*x + b; a = 2/(max-min), b = (negmin-max)/(max-min)
                v = nc.vector
                GRm = sm_pool.tile([1, 2], F32, tag=f"GRm{s}", name=f"GRm{s}")
                v.tensor_reduce(GRm[:], GRt[:].rearrange("o (r j) -> o j r", r=8),
                                AX.X, OP.max)
                den = sm_pool.tile([1, 1], F32, tag=f"den{s}")
                v.tensor_add(den[:], GRm[:, 0:1], GRm[:, 1:2])
                rden = sm_pool.tile([1, 1], F32, tag=f"rden{s}")
                v.reciprocal(rden[:], den[:])
                S2 = sm_pool.tile([1, 2], F32, tag=f"S2{s}")
                v.tensor_scalar_mul(S2[:, 0:1], rden[:], 2.0)
                dif = sm_pool.tile([1, 1], F32, tag=f"dif{s}")
                v.tensor_sub(dif[:], GRm[:, 1:2], GRm[:, 0:1])
                v.tensor_mul(S2[:, 1:2], dif[:], rden[:])
                nc.gpsimd.partition_broadcast(SCB[:, s:s + 1], S2[:, 0:1])
                nc.gpsimd.partition_broadcast(SCB[:, S + s:S + s + 1],
                                              S2[:, 1:2])

            # ---------------- phase B: streams + PE sums ----------------
            # Software-pipelined "phase skew": P1 = ops[0:16] (t1..u16),
            # P2 = ops[16:32] (u16-dependent leaves). Window k emits the ACT
            # anchor chain of tile k, then P2 of tile k-1 (inputs all ready),
            # then the rest of P1 of tile k. PE consumes P2(k-1) mms while
            # tile k's chain fills, so it never starves.
            NP1 = 16
            assert ops[NP1 - 1][0] == "u16"
            tile_streams = {}
            seg_ps = {}

            # per-tag buffer counts (cross-window readers need 2)
            TAG_BUFS = {"t1": 2, "a3": 2, "a5": 2, "a7": 2, "a9": 2,
                        "a11": 2, "a13": 2, "a15": 2,
                        "u2": 2, "u4": 2, "u8": 2, "u16": 2,
                        "e12": 2, "e6": 1, "e14": 2}

            def stream_tile(nm):
                if (nm.startswith("a") and int(nm[1:]) >= 17) or \
                   nm in ("e10", "e18", "e20", "e22", "e24", "e26", "e28",
                          "e30", "e32"):
                    return j_pool.tile([128, FT], F16, tag="junk", name="junk")
                pool = an_pool if nm[0] in "ue" else pr_pool
                return pool.tile([128, FT], F16, tag=nm, bufs=TAG_BUFS[nm],
                                 name=nm)

            def mm(ps, idx, v, start=False, stop=False):
                for ch in range(4):
                    nc.tensor.matmul(
                        ps[0:NRAW, 512 * ch:512 * (ch + 1)],
                        Emat[:, idx, :], v[:, 512 * ch:512 * (ch + 1)],
                        start=start, stop=stop,
                        skip_group_check=True)

            def emit_op(st, s, nm, kind, s1, s2, xbf=None):
                t = stream_tile(nm)
                if kind == "tsp":
                    nc.scalar.activation(t[:], xbf, AF.Identity,
                                         bias=SCB[:, S + s:S + s + 1],
                                         scale=SCB[:, s:s + 1])
                elif kind == "sq":
                    bv = round(float(s2[1]), 9)
                    bias_ap = bias_t[bv][:, 0:1] if bv != 0.0 else 0.0
                    nc.scalar.activation(t[:], st[s1][:], AF.Square,
                                         bias=bias_ap, scale=float(s2[0]))
                else:
                    eng = nc.vector if kind == "dve" else nc.gpsimd
                    eng.tensor_mul(t[:], st[s1][:], st[s2][:])
                st[nm] = t
                return t

            def emit_drain(s):
                nc.vector.tensor_reduce(
                    Sacc[:, s, :],
                    seg_ps[s][0:NRAW].rearrange("k (c i) -> k c i", c=C),
                    AX.X, OP.add)

            OP_IDX = {nm: i for i, (nm, _, _, _) in enumerate(ops)}
            # matmul emission order ~ stream-readiness order: interleaves
            # prev-tile P2 leaves ("p") with current-tile chain ("c") and
            # rest-of-P1 ("r") so the in-order PE never starves.
            MM_ORDER = [
                ("p", "a17"), ("c", "t1"), ("p", "a19"), ("p", "a21"),
                ("c", "u2"), ("p", "e20"), ("p", "a23"), ("p", "a25"),
                ("c", "u4"), ("p", "a27"), ("p", "a29"), ("c", "u8"),
                ("p", "a31"), ("p", "e30"), ("p", "e22"), ("c", "u16"),
                ("r", "a3"), ("r", "a5"), ("p", "e18"), ("r", "a7"),
                ("r", "e6"), ("p", "e24"), ("r", "a9"), ("r", "a11"),
                ("p", "e26"), ("r", "a13"), ("r", "e12"), ("r", "a15"),
                ("p", "e28"), ("r", "e10"), ("r", "e14"), ("p", "e32"),
            ]
            assert len(MM_ORDER) == 32

            # ---------------- combine + out ----------------
            def combine(s, split=False):
                # rho[c] = sum_i q[i,c]*Sacc[i,c] via tiny f32 matmul into the
                # spare PSUM partitions (64..127) of the segment accumulator.
                prod_ = sm_pool.tile([NRAW, C], F32, tag=f"pr{s}")
                nc.vector.tensor_mul(prod_[:], Sacc[:, s, :],
                                     q_sb[:, C * s:C * (s + 1)])
                ps = seg_ps[s]
                nc.tensor.matmul(ps[64:128, s:s + 1], prod_[:], ones32f[:],
                                 start=True, stop=True, skip_group_check=True)
                rt = sm_pool.tile([64, 1], F32, tag=f"rt{s}")
                nc.vector.tensor_add(rt[:], ps[64:128, s:s + 1],
                                     off_sb[:, s:s + 1])
                # broadcast build: [64, 4096] f32 = rho bias (scale=0 ignores
                # the input, so read the output tile itself)
                ot = o_pool.tile([64, 2 * FT], F32, tag="ot", bufs=1)
                if split:
                    # latency-critical tail: build halves on ACT + DVE
                    nc.scalar.activation(ot[:, 0:FT], zeros_ot[:, 0:FT],
                                         AF.Identity,
                                         bias=rt[:, 0:1], scale=0.0)
                    nc.vector.tensor_scalar(ot[:, FT:], zeros_ot[:, FT:], 0.0,
                                            rt[:, 0:1], OP.mult, OP.add)
                else:
                    nc.scalar.activation(ot[:], zeros_ot[:], AF.Identity,
                                         bias=rt[:, 0:1], scale=0.0)
                return ot

            def out_dma(s, ot, t, eng=None):
                h, tt_ = s // 2, (s % 2) * 4 + t
                (eng or nc.sync).dma_start(
                    outr[h, :, bass.ts(tt_, 2 * FT)], ot[:])

            # ---------------- schedule ----------------
            # head minmax via cheap-DMA layout tiles (512B runs, ~2x faster
            # loads), then reload segment 0 in compute layout behind the
            # collective; head tiles rotate through the same xb pool bufs.
            s0 = 0
            for g in range(NQ):
                hx = xb_pool.tile([128, C, 32], F32, tag="xb",
                                  name=f"hx{g}")
                nc.sync.dma_start(
                    hx[:].rearrange("p c i -> p (c i)"), src_ap_cg(s0, g))
                minmax_dve(s0, g, tile=hx)
                minmax_pool(s0, g, tile=hx)
            GR0 = phaseA_fold(s0)
            for qq in range(NQ):
                load_tile(s0, qq)

            # PE p-state warmup while the collective is in flight
            NWARM = 38
            ps_w = ps_pool.tile([128, FT], F32, tag="ps", name="ps_w")
            for wi in range(NWARM):
                for ch in range(4):
                    nc.tensor.matmul(
                        ps_w[0:NRAW, 512 * ch:512 * (ch + 1)],
                        Emat[:, 0, :], zeros16[:, :],
                        start=(wi == 0), stop=(wi == NWARM - 1),
                        skip_group_check=True)
            phaseA_calc(s0, GR0)

            GR_next = [None]
            out_work = {}
            NT = S * NQ

            pending_drain = [None]
            pending_combine = [None]
            HOOK_AT = 15   # after all P2-DVE entries in MM_ORDER

            for k in range(NT + 1):
                s, q = divmod(k, NQ) if k < NT else (None, None)
                sp, qp = divmod(k - 1, NQ) if k >= 1 else (None, None)
                s_next = (s + 1 if s is not None and s + 1 < S else None)

                def dve_aux():
                    # DVE aux early inside the P2-DVE run (PE has buffer
                    # there); keeps the DVE second half free for P1 streams
                    if s_next is not None:
                        if q == 0:
                            minmax_dve(s_next, 0)
                        elif q == 1:
                            minmax_dve(s_next, 2)
                        elif q == 2:
                            minmax_dve(s_next, 3)

                def dve_aux2():
                    if pending_drain[0] is not None:
                        sd = pending_drain[0]
                        pending_drain[0] = None
                        emit_drain(sd)
                        pending_combine[0] = sd

                def hook():
                    if pending_combine[0] is not None:
                        sc = pending_combine[0]
                        pending_combine[0] = None
                        out_work[sc] = combine(sc)
                    if s_next is not None and q == 3:
                        phaseA_calc(s_next, GR_next[0])
                    # out DMAs for the previously combined segment
                    if s is not None and s > 0 and q in (2, 3) \
                            and (s - 1) in out_work:
                        otp = out_work[s - 1]
                        for t in range(2):
                            out_dma(s - 1, otp, (q - 2) * 2 + t)

                def tail_aux():
                    if s_next is not None:
                        if q == 0:
                            minmax_dve(s_next, 1)
                            minmax_pool(s_next, 0)
                            minmax_pool(s_next, 1)
                        elif q == 1:
                            minmax_pool(s_next, 2)
                        elif q == 2:
                            minmax_pool(s_next, 3)
                            GR_next[0] = phaseA_fold(s_next)

                # --- loads first (independent SP-queue work) ---
                if s_next is not None:
                    if q == 0:
                        load_tile(s_next, 0)
                        load_tile(s_next, 1)
                    elif q == 1:
                        load_tile(s_next, 2)
                        load_tile(s_next, 3)

                # --- (op, matmuls) pairs in readiness order ---
                if k < NT:
                    if q == 0:
                        seg_ps[s] = ps_pool.tile([128, FT], F32, tag="ps",
                                                 name=f"ps{s}")
                    xb = x_tiles.pop((s, q))
                    xbf = xb[:].rearrange("p c i -> p (c i)")
                    tile_streams[(s, q)] = {}
                for mi, (src, nm) in enumerate(MM_ORDER):
                    if mi == 4:
                        dve_aux()
                    if mi == 9:
                        dve_aux2()
                    if mi == HOOK_AT:
                        hook()

                    i = OP_IDX[nm]
                    _, kind, s1, s2 = ops[i]
                    if src == "p":
                        if k >= 1:
                            st = tile_streams[(sp, qp)]
                            t = emit_op(st, sp, nm, kind, s1, s2)
                            mm(seg_ps[sp], i, t,
                               stop=(nm == "e32" and qp == NQ - 1))
                    else:
                        if k < NT:
                            st = tile_streams[(s, q)]
                            t = emit_op(st, s, nm, kind, s1, s2, xbf=xbf)
                            mm(seg_ps[s], i, t,
                               start=(nm == "t1" and q == 0))
                            if k == 0:
                                # window 0 is production-paced: fillers that
                                # read the fresh stream execute in the gaps,
                                # keeping the PE p-state hot
                                for _ in range(2):
                                    nc.tensor.matmul(
                                        ps_w[0:NRAW, 0:512], Emat[:, 0, :],
                                        t[:, 0:512], start=True, stop=True,
                                        skip_group_check=True)
                if k < NT:
                    tail_aux()
                if k >= 1:
                    del tile_streams[(sp, qp)]
                    if qp == NQ - 1:
                        pending_drain[0] = sp

            # tail: drain + combine + outs of the last segment on 4 queues
            emit_drain(S - 1)
            ot = combine(S - 1, split=True)
            for t, eng in enumerate((nc.sync, nc.gpsimd, nc.scalar, nc.sync)):
                out_dma(S - 1, ot, t, eng=eng)

    nc.compile()
    return nc


_NC_CACHE = {}


def _get_nc():
    if "nc" not in _NC_CACHE:
        _NC_CACHE["nc"] = build_kernel()
    return _NC_CACHE["nc"]


def kernel(x: np.ndarray, chebyshev_weights: np.ndarray, **run_kwargs) -> np.ndarray:
    x = np.ascontiguousarray(np.asarray(x, dtype=np.float32))
    W = np.asarray(chebyshev_weights, dtype=np.float32)
    assert x.shape == (B, C, L), x.shape
    _, R = build_dag()
    q_dev, off_dev = host_weight_transform(W, R)

    nc = _get_nc()
    in_maps = [
        {"x": x[b], "q": q_dev, "off": off_dev}
        for b in range(NCORES)
    ]
    res = run_bass_kernel_spmd(nc, in_maps, core_ids=list(range(NCORES)),
                               **run_kwargs)
    out = np.stack([res.results[b]["out"] for b in range(NCORES)], axis=0)
    kernel.last_results = res
    return out
